# revision 13
# baseline (speedup 1.0000x reference)
"""Sparse multi-head attention (ViT-style, 577 tokens, 12 heads) on 8 TRN2
NeuronCores.

Sharding: pure data-parallel over batch. Each core gets 8 of the 64 batch
items: 4 from the "large" half (full 12-head attention) and 4 from the
"small" half (compressed: heads 6..11 of q/k/v are statically zero, so only
6 heads + a 384x384 projection are computed). Co-sharding large/small
halves balances per-core compute. No collectives are needed.

Per-item dataflow (everything stays in the transposed domain so no
intermediate ever needs a device transpose except the initial x -> xT):

  x[577,768] --PE-transpose--> xT[c,n]
  qT,kT[o,n] = Wqkv^T-stationary matmuls over xT     (q pre-scaled by D^-0.5)
  v[n,o]     = xT-stationary matmuls over Wv^T, plus a ones column (aug)
  S^T[m,n]   = kT-stationary over qT (per head, K=64)
  P^T        = exp(S^T)  (scalar engine, PSUM->SBUF, bf16; softmax max-shift
               skipped: logits are O(1) by construction)
  aoT[d,n]   = v_aug^T @ P^T  -> row 64 holds the softmax denominators
  normalize via reciprocal + K=1 broadcast matmul
  y[n,oc]    = aoT-stationary over proj_w^T, + bias, DMA out.

Matmuls run as float32r (full PE rate at free-dim >= 256) on fp32 data;
only P^T and v are bf16 (flash-attention-style precision).
"""

import ml_dtypes
import numpy as np
from contextlib import ExitStack

import concourse.bass as bass
import concourse.tile as tile
from concourse import bacc, mybir
from concourse import bass2jax as _b2j
from concourse.bass_utils import run_bass_kernel_spmd
from concourse.masks import make_identity


def _run_bass_via_pjrt_presharded(nc, in_maps, n_cores):
    """Drop-in replacement for bass2jax.run_bass_via_pjrt (multi-core path).

    The stock version concatenates per-core inputs into one host array and
    lets jax reshard it onto the mesh; on the neuron PJRT backend that
    resharding lowers to a compiled "scatter" program which, for ~100MB
    inputs, dies in neuronx-cc codegen (16-bit semaphore_wait_value
    overflow). Here each per-core shard is device_put directly onto its
    device and the global array is assembled zero-copy, so the jitted body
    sees correctly-sharded operands and no data-movement program exists.
    """
    import jax

    _b2j.install_neuronx_cc_hook()
    assert nc.dbg_addr is None and nc.partition_id_tensor is None

    from jax.experimental.shard_map import shard_map
    from jax.sharding import Mesh, NamedSharding, PartitionSpec

    in_names, out_names, out_avals, zero_shapes = [], [], [], []
    for alloc in nc.m.functions[0].allocations:
        if not isinstance(alloc, mybir.MemoryLocationSet):
            continue
        name = alloc.memorylocations[0].name
        if alloc.kind == "ExternalInput":
            in_names.append(name)
        elif alloc.kind == "ExternalOutput":
            shape = tuple(alloc.tensor_shape)
            dtype = mybir.dt.np(alloc.dtype)
            out_names.append(name)
            out_avals.append(jax.core.ShapedArray(shape, dtype))
            zero_shapes.append((shape, dtype))
    n_params = len(in_names)
    n_outs = len(out_names)
    all_names = in_names + out_names
    donate = tuple(range(n_params, n_params + n_outs))

    def _body(*args):
        outs = _b2j._bass_exec_p.bind(
            *args,
            out_avals=tuple(out_avals),
            in_names=tuple(all_names),
            out_names=tuple(out_names),
            lowering_input_output_aliases=(),
            sim_require_finite=True,
            sim_require_nnan=True,
            nc=nc,
        )
        return tuple(outs)

    devices = jax.devices()[:n_cores]
    mesh = Mesh(np.asarray(devices), ("core",))
    sharding = NamedSharding(mesh, PartitionSpec("core"))

    def make_global(shards):
        s0 = np.asarray(shards[0])
        gshape = (n_cores * s0.shape[0], *s0.shape[1:])
        parts = [
            jax.device_put(np.ascontiguousarray(shards[c]), devices[c])
            for c in range(n_cores)
        ]
        return jax.make_array_from_single_device_arrays(gshape, sharding, parts)

    global_ins = [make_global([m[nm] for m in in_maps]) for nm in in_names]
    global_zeros = [
        make_global([np.zeros(shape, dtype)] * n_cores)
        for shape, dtype in zero_shapes
    ]

    sharded = jax.jit(
        shard_map(_body, mesh=mesh, in_specs=(PartitionSpec("core"),) * (n_params + n_outs),
                  out_specs=(PartitionSpec("core"),) * n_outs, check_rep=False),
        donate_argnums=donate,
        keep_unused=True,
    )
    out_arrs = sharded(*global_ins, *global_zeros)

    results = [dict() for _ in range(n_cores)]
    for i, name in enumerate(out_names):
        arr = out_arrs[i]
        per = {s.index[0].start or 0: np.asarray(s.data) for s in arr.addressable_shards}
        step = out_avals[i].shape[0]
        for c in range(n_cores):
            results[c][name] = per[c * step]
    return results


def _patched_run_bass_via_pjrt(nc, in_maps, n_cores):
    if n_cores > 1 and nc.partition_id_tensor is None and nc.dbg_addr is None:
        return _run_bass_via_pjrt_presharded(nc, in_maps, n_cores)
    return _orig_run_bass_via_pjrt(nc, in_maps, n_cores)


_orig_run_bass_via_pjrt = _b2j.run_bass_via_pjrt
_b2j.run_bass_via_pjrt = _patched_run_bass_via_pjrt

P = 128
N = 577
C = 768
H = 12
D = 64
NCH = 5           # n (token) chunks: 4*128 + 65
CCH = 6           # c chunks: 768 / 128
NTAIL = N - 4 * P  # 65
F0, F1 = 290, 288  # n free-dim halves, padded n=578: fp32r needs EVEN free sizes
HALVES = ((0, F0), (F0, F1))
ITEMS = 8
NCORES = 8

f32 = mybir.dt.float32
f32r = mybir.dt.float32r
bf16 = mybir.dt.bfloat16


def _rows(nch):
    return NTAIL if nch == NCH - 1 else P


def _emit(ctx, tc, x_ext, wq_ext, pw_ext, pb_ext, sel_ext, out_ext):
    nc = tc.nc

    const_pool = ctx.enter_context(tc.tile_pool(name="const", bufs=1))
    wpool = ctx.enter_context(tc.tile_pool(name="weights", bufs=1))
    xpool = ctx.enter_context(tc.tile_pool(name="xchunk", bufs=3))
    xtpool = ctx.enter_context(tc.tile_pool(name="xt", bufs=2))
    qkpool = ctx.enter_context(tc.tile_pool(name="qkt", bufs=1))
    vpool = ctx.enter_context(tc.tile_pool(name="vnat", bufs=1))
    epool = ctx.enter_context(tc.tile_pool(name="exps", bufs=3))
    aopool = ctx.enter_context(tc.tile_pool(name="aot", bufs=1))
    ypool = ctx.enter_context(tc.tile_pool(name="ychunk", bufs=3))
    spool = ctx.enter_context(tc.tile_pool(name="norm", bufs=2))
    aoupool = ctx.enter_context(tc.tile_pool(name="aou", bufs=1))
    ps = ctx.enter_context(tc.tile_pool(name="ps", bufs=4, space="PSUM"))

    def ps_tile(name):
        return ps.tile([P, 2, 512], f32, tag="ps", name=name)

    # ---- constants / weights (resident) ----
    ident = const_pool.tile([P, P], f32, name="ident")
    make_identity(nc, ident)

    ones_f32 = const_pool.tile([1, P], f32, name="ones_f32")
    nc.gpsimd.memset(ones_f32[:], 1.0)
    ones_row = const_pool.tile([1, P], f32r, name="ones_row")
    nc.vector.tensor_copy(ones_row[:], ones_f32[:])

    sel = const_pool.tile([12, CCH, P], bf16, name="sel")
    nc.sync.dma_start(sel[:], sel_ext[:])

    wq_sb = wpool.tile([P, CCH, 3 * C], bf16, name="wq_sb")
    nc.sync.dma_start(wq_sb[:], wq_ext.rearrange("(co p) o -> p co o", p=P))
    pw_sb = wpool.tile([P, CCH, C], bf16, name="pw_sb")
    nc.sync.dma_start(pw_sb[:], pw_ext.rearrange("(co p) o -> p co o", p=P))
    pb_sb = const_pool.tile([1, C], f32r, name="pb_sb")
    nc.sync.dma_start(pb_sb[:], pb_ext[None, :])

    # bias broadcast across partitions: [128, 768] = ones[128,1] @ pb[1,768]
    bias_sb = wpool.tile([P, C], f32, name="bias_sb")
    psb0 = ps_tile("ps_bias")
    for j in range(2):
        nc.tensor.matmul(
            psb0[:, j, 0:384],
            lhsT=ones_row[0:1, :],
            rhs=pb_sb[0:1, j * 384:(j + 1) * 384],
            start=True, stop=True,
        )
        nc.vector.tensor_copy(bias_sb[:, j * 384:(j + 1) * 384], psb0[:, j, 0:384])

    # ---- per-item pipeline ----
    for it in range(ITEMS):
        small = it >= ITEMS // 2
        Heff = H // 2 if small else H
        # o-chunk ids within the q|k layout of qkT (q: 0..5, k: 6..11)
        qk_chunks = ([0, 1, 2, 6, 7, 8] if small else list(range(12)))
        CCH_ao = CCH // 2 if small else CCH   # proj contraction chunks
        NJ = 1 if small else 2                # 384-wide column groups

        # Phase A: load x chunks, transpose to xT [c-part, n-free]
        xT = xtpool.tile([P, CCH, 640], bf16, name="xT")
        nc.gpsimd.memset(xT[:, :, N], 0.0)
        for nch in range(NCH):
            rows = _rows(nch)
            xc = xpool.tile([P, C], f32, name="xc")
            nc.sync.dma_start(xc[0:rows, :], x_ext[it, nch * P:nch * P + rows, :])
            for cc0 in range(0, CCH, 2):
                pst = ps_tile("ps_t")
                for j in (0, 1):
                    cc = cc0 + j
                    nc.tensor.transpose(
                        pst[:, j, 0:rows],
                        xc[0:rows, cc * P:(cc + 1) * P],
                        ident[0:rows, 0:rows],
                    )
                nc.vector.tensor_copy(
                    xT[:, cc0:cc0 + 2, nch * P:nch * P + rows],
                    pst[:, :, 0:rows],
                )

        # Phase B: qT / kT (transposed outputs) for needed o-chunks
        qkT = qkpool.tile([P, 12, N + 1], bf16, name="qkT")
        for oc in qk_chunks:
            wcol = oc * P if oc < 6 else C + (oc - 6) * P
            pqk = ps_tile("ps_qk")
            for j, (n0, nsz) in enumerate(HALVES):
                for cc in range(CCH):
                    nc.tensor.matmul(
                        pqk[:, j, 0:nsz],
                        lhsT=wq_sb[:, cc, wcol:wcol + P],
                        rhs=xT[:, cc, n0:n0 + nsz],
                        start=(cc == 0), stop=(cc == CCH - 1),
                    )
                nc.vector.tensor_copy(qkT[:, oc, n0:n0 + nsz], pqk[:, j, 0:nsz])

        # Phase C: v natural [n-part, (h,d)-free] in bf16 with ones column
        v_nat = vpool.tile([P, NCH, H, D + 1], bf16, name="v_nat")
        nc.gpsimd.memset(v_nat[:, :, :, D], 1.0)
        for nch in range(NCH):
            rows = _rows(nch)
            pv = ps_tile("ps_v")
            for j in range(NJ):
                vcol = 2 * C + j * 384
                for cc in range(CCH):
                    nc.tensor.matmul(
                        pv[0:rows, j, 0:384],
                        lhsT=xT[:, cc, nch * P:nch * P + rows],
                        rhs=wq_sb[:, cc, vcol:vcol + 384],
                        start=(cc == 0), stop=(cc == CCH - 1),
                    )
                nc.vector.tensor_copy(
                    v_nat[0:rows, nch, j * 6:(j + 1) * 6, 0:D],
                    pv[0:rows, j, 0:384].rearrange("p (h d) -> p h d", h=6),
                )

        # Phase D/E: scores^T (head-pairs packed via tile_position) -> exp
        # -> AV with ones-augmented v (row 64 = softmax denominators).
        # Denominator rows collect in dsum; one batched reciprocal per item,
        # then K=1 broadcast matmuls + DVE multiply normalize into aoT.
        aoT = aopool.tile([P, CCH, N + 1], bf16, name="aoT")
        aoU = aoupool.tile([P, CCH, 2, F0], f32, name="aoU")
        dsum = spool.tile([12, 2, F0], f32, name="dsum")
        for hp in range(Heff // 2):
            kch = 6 + hp
            pse = [None, None]
            expS = [None, None]
            for hh in (0, 1):
                expS[hh] = epool.tile([P, NCH, 2, F0], bf16, tag="expS",
                                      name=f"expS{hh}")
            for mch in range(NCH):
                mrows = _rows(mch)
                for hh in (0, 1):
                    hrow = hh * D
                    pse[hh] = ps_tile(f"ps_s{hh}")
                    for j, (n0, nsz) in enumerate(HALVES):
                        nc.tensor.matmul(
                            pse[hh][0:mrows, j, 0:nsz],
                            lhsT=qkT[hrow:hrow + D, kch, mch * P:mch * P + mrows],
                            rhs=qkT[hrow:hrow + D, hp, n0:n0 + nsz],
                            start=True, stop=True,
                            tile_position=(hrow, 0),
                        )
                for hh in (0, 1):
                    nc.scalar.activation(
                        expS[hh][0:mrows, mch, :, :],
                        pse[hh][0:mrows, :, 0:F0],
                        mybir.ActivationFunctionType.Exp,
                    )
            for hh in (0, 1):
                h = 2 * hp + hh
                po = ps_tile("ps_o")
                for j, (n0, nsz) in enumerate(HALVES):
                    for mch in range(NCH):
                        mrows = _rows(mch)
                        nc.tensor.matmul(
                            po[0:D + 1, j, 0:nsz],
                            lhsT=v_nat[0:mrows, mch, h, :],
                            rhs=expS[hh][0:mrows, mch, j, 0:nsz],
                            start=(mch == 0), stop=(mch == NCH - 1),
                        )
                arow = hh * D
                nc.vector.tensor_copy(aoU[arow:arow + D, hp, :, :],
                                      po[0:D, :, 0:F0])
                dstage = spool.tile([1, 2, F0], f32, name="dstage")
                nc.vector.tensor_copy(dstage[0:1, :, :], po[D:D + 1, :, 0:F0])
                nc.sync.dma_start(dsum[h:h + 1, :, :], dstage[0:1, :, :])

        drecip = spool.tile([12, 2, F0], bf16, name="drecip")
        with nc.allow_low_precision(reason="softmax recip bcast via bf16 matmul"):
            nc.vector.reciprocal(drecip[0:Heff, :, :], dsum[0:Heff, :, :])
        for c in range(Heff // 2):
            pbc = ps_tile("ps_bc")
            for j, (n0, nsz) in enumerate(HALVES):
                nc.tensor.matmul(
                    pbc[:, j, 0:nsz],
                    lhsT=sel[0:12, c, :],
                    rhs=drecip[0:12, j, 0:nsz],
                    start=True, stop=True,
                )
            for j, (n0, nsz) in enumerate(HALVES):
                nc.vector.tensor_mul(
                    aoT[:, c, n0:n0 + nsz],
                    aoU[:, c, j, 0:nsz],
                    pbc[:, j, 0:nsz],
                )

        # Phase F: projection + bias (+ zero tail channels for small), DMA out
        for nch in range(NCH):
            rows = _rows(nch)
            psy = ps_tile("ps_y")
            yc = ypool.tile([P, C], f32, name="yc")
            if small:
                nc.gpsimd.memset(yc[0:rows, 384:768], 0.0)
            for j in range(NJ):
                o0 = j * 384
                for cc in range(CCH_ao):
                    nc.tensor.matmul(
                        psy[0:rows, j, 0:384],
                        lhsT=aoT[:, cc, nch * P:nch * P + rows],
                        rhs=pw_sb[:, cc, o0:o0 + 384],
                        start=(cc == 0), stop=(cc == CCH_ao - 1),
                    )
                nc.vector.tensor_add(
                    yc[0:rows, o0:o0 + 384],
                    psy[0:rows, j, 0:384],
                    bias_sb[0:rows, o0:o0 + 384],
                )
            nc.sync.dma_start(out_ext[it, nch * P:nch * P + rows, :], yc[0:rows, :])


_GRAPH = None


def _get_graph():
    global _GRAPH
    if _GRAPH is None:
        nc = bacc.Bacc("TRN2", target_bir_lowering=False, debug=False,
                       num_devices=NCORES)
        x_ext = nc.dram_tensor("x", [ITEMS, N, C], f32, kind="ExternalInput").ap()
        wq_ext = nc.dram_tensor("wq", [C, 3 * C], bf16, kind="ExternalInput").ap()
        pw_ext = nc.dram_tensor("pw", [C, C], bf16, kind="ExternalInput").ap()
        pb_ext = nc.dram_tensor("pb", [C], f32r, kind="ExternalInput").ap()
        sel_ext = nc.dram_tensor("sel", [12, CCH, P], bf16, kind="ExternalInput").ap()
        out_ext = nc.dram_tensor("out", [ITEMS, N, C], f32, kind="ExternalOutput").ap()
        with tile.TileContext(nc) as tc:
            with ExitStack() as ctx:
                _emit(ctx, tc, x_ext, wq_ext, pw_ext, pb_ext, sel_ext, out_ext)
        nc.finalize()
        _GRAPH = nc
    return _GRAPH


LAST_RESULTS = None


def kernel(x, qkv_w, proj_w, proj_b, _trace=False):
    global LAST_RESULTS
    x = np.asarray(x, dtype=np.float32)
    wq = np.array(qkv_w, dtype=np.float32)          # copy; rows 0:C are q
    wq[0:C] *= D ** -0.5                            # fold attention scale into Wq
    wqT = np.ascontiguousarray(wq.T).astype(ml_dtypes.bfloat16)   # [C, 3C]
    pwT = np.ascontiguousarray(
        np.asarray(proj_w, dtype=np.float32).T).astype(ml_dtypes.bfloat16)
    pb = np.ascontiguousarray(np.asarray(proj_b, dtype=np.float32))
    sel_np = np.zeros((12, CCH, P), dtype=ml_dtypes.bfloat16)
    for c in range(CCH):
        sel_np[2 * c, c, 0:D] = 1
        sel_np[2 * c + 1, c, D:P] = 1

    nc = _get_graph()
    in_maps = []
    half = x.shape[0] // 2  # 32
    per = half // NCORES    # 4
    for c in range(NCORES):
        xs = np.concatenate(
            [x[per * c:per * (c + 1)], x[half + per * c:half + per * (c + 1)]],
            axis=0,
        )
        in_maps.append({
            "x": np.ascontiguousarray(xs),
            "wq": wqT,
            "pw": pwT,
            "pb": pb,
            "sel": sel_np,
        })

    res = run_bass_kernel_spmd(nc, in_maps, core_ids=list(range(NCORES)),
                               trace=_trace)
    LAST_RESULTS = res

    out = np.empty((x.shape[0], N, C), dtype=np.float32)
    for c in range(NCORES):
        o = res.results[c]["out"]
        out[per * c:per * (c + 1)] = o[0:per]
        out[half + per * c:half + per * (c + 1)] = o[per:2 * per]
    return out


# revision 14
# speedup vs baseline: 1.0124x; 1.0124x over previous
"""Sparse multi-head attention (ViT-style, 577 tokens, 12 heads) on 8 TRN2
NeuronCores.

Sharding: pure data-parallel over batch. Each core gets 8 of the 64 batch
items: 4 from the "large" half (full 12-head attention) and 4 from the
"small" half (compressed: heads 6..11 of q/k/v are statically zero, so only
6 heads + a 384x384 projection are computed). Co-sharding large/small
halves balances per-core compute. No collectives are needed.

Per-item dataflow (everything stays in the transposed domain so no
intermediate ever needs a device transpose except the initial x -> xT):

  x[577,768] --PE-transpose--> xT[c,n]
  qT,kT[o,n] = Wqkv^T-stationary matmuls over xT     (q pre-scaled by D^-0.5)
  v[n,o]     = xT-stationary matmuls over Wv^T, plus a ones column (aug)
  S^T[m,n]   = kT-stationary over qT (per head, K=64)
  P^T        = exp(S^T)  (scalar engine, PSUM->SBUF, bf16; softmax max-shift
               skipped: logits are O(1) by construction)
  aoT[d,n]   = v_aug^T @ P^T  -> row 64 holds the softmax denominators
  normalize via reciprocal + K=1 broadcast matmul
  y[n,oc]    = aoT-stationary over proj_w^T, + bias, DMA out.

Matmuls run as float32r (full PE rate at free-dim >= 256) on fp32 data;
only P^T and v are bf16 (flash-attention-style precision).
"""

import ml_dtypes
import numpy as np
from contextlib import ExitStack

import concourse.bass as bass
import concourse.tile as tile
from concourse import bacc, mybir
from concourse import bass2jax as _b2j
from concourse.bass_utils import run_bass_kernel_spmd
from concourse.masks import make_identity


def _run_bass_via_pjrt_presharded(nc, in_maps, n_cores):
    """Drop-in replacement for bass2jax.run_bass_via_pjrt (multi-core path).

    The stock version concatenates per-core inputs into one host array and
    lets jax reshard it onto the mesh; on the neuron PJRT backend that
    resharding lowers to a compiled "scatter" program which, for ~100MB
    inputs, dies in neuronx-cc codegen (16-bit semaphore_wait_value
    overflow). Here each per-core shard is device_put directly onto its
    device and the global array is assembled zero-copy, so the jitted body
    sees correctly-sharded operands and no data-movement program exists.
    """
    import jax

    _b2j.install_neuronx_cc_hook()
    assert nc.dbg_addr is None and nc.partition_id_tensor is None

    from jax.experimental.shard_map import shard_map
    from jax.sharding import Mesh, NamedSharding, PartitionSpec

    in_names, out_names, out_avals, zero_shapes = [], [], [], []
    for alloc in nc.m.functions[0].allocations:
        if not isinstance(alloc, mybir.MemoryLocationSet):
            continue
        name = alloc.memorylocations[0].name
        if alloc.kind == "ExternalInput":
            in_names.append(name)
        elif alloc.kind == "ExternalOutput":
            shape = tuple(alloc.tensor_shape)
            dtype = mybir.dt.np(alloc.dtype)
            out_names.append(name)
            out_avals.append(jax.core.ShapedArray(shape, dtype))
            zero_shapes.append((shape, dtype))
    n_params = len(in_names)
    n_outs = len(out_names)
    all_names = in_names + out_names
    donate = tuple(range(n_params, n_params + n_outs))

    def _body(*args):
        outs = _b2j._bass_exec_p.bind(
            *args,
            out_avals=tuple(out_avals),
            in_names=tuple(all_names),
            out_names=tuple(out_names),
            lowering_input_output_aliases=(),
            sim_require_finite=True,
            sim_require_nnan=True,
            nc=nc,
        )
        return tuple(outs)

    devices = jax.devices()[:n_cores]
    mesh = Mesh(np.asarray(devices), ("core",))
    sharding = NamedSharding(mesh, PartitionSpec("core"))

    def make_global(shards):
        s0 = np.asarray(shards[0])
        gshape = (n_cores * s0.shape[0], *s0.shape[1:])
        parts = [
            jax.device_put(np.ascontiguousarray(shards[c]), devices[c])
            for c in range(n_cores)
        ]
        return jax.make_array_from_single_device_arrays(gshape, sharding, parts)

    global_ins = [make_global([m[nm] for m in in_maps]) for nm in in_names]
    global_zeros = [
        make_global([np.zeros(shape, dtype)] * n_cores)
        for shape, dtype in zero_shapes
    ]

    sharded = jax.jit(
        shard_map(_body, mesh=mesh, in_specs=(PartitionSpec("core"),) * (n_params + n_outs),
                  out_specs=(PartitionSpec("core"),) * n_outs, check_rep=False),
        donate_argnums=donate,
        keep_unused=True,
    )
    out_arrs = sharded(*global_ins, *global_zeros)

    results = [dict() for _ in range(n_cores)]
    for i, name in enumerate(out_names):
        arr = out_arrs[i]
        per = {s.index[0].start or 0: np.asarray(s.data) for s in arr.addressable_shards}
        step = out_avals[i].shape[0]
        for c in range(n_cores):
            results[c][name] = per[c * step]
    return results


def _patched_run_bass_via_pjrt(nc, in_maps, n_cores):
    if n_cores > 1 and nc.partition_id_tensor is None and nc.dbg_addr is None:
        return _run_bass_via_pjrt_presharded(nc, in_maps, n_cores)
    return _orig_run_bass_via_pjrt(nc, in_maps, n_cores)


_orig_run_bass_via_pjrt = _b2j.run_bass_via_pjrt
_b2j.run_bass_via_pjrt = _patched_run_bass_via_pjrt

P = 128
N = 577
C = 768
H = 12
D = 64
NCH = 5           # n (token) chunks: 4*128 + 65
CCH = 6           # c chunks: 768 / 128
NTAIL = N - 4 * P  # 65
F0, F1 = 290, 288  # n free-dim halves, padded n=578: fp32r needs EVEN free sizes
HALVES = ((0, F0), (F0, F1))
ITEMS = 8
NCORES = 8

f32 = mybir.dt.float32
f32r = mybir.dt.float32r
bf16 = mybir.dt.bfloat16


def _rows(nch):
    return NTAIL if nch == NCH - 1 else P


def _mcols(nch):
    """lhsT column count for an n-chunk: pad the 65-tail to 66 (even M is
    measurably faster on the PE); the extra output partition is discarded."""
    return NTAIL + 1 if nch == NCH - 1 else P


def _emit(ctx, tc, x_ext, wq_ext, pw_ext, pb_ext, sel_ext, out_ext):
    nc = tc.nc

    const_pool = ctx.enter_context(tc.tile_pool(name="const", bufs=1))
    wpool = ctx.enter_context(tc.tile_pool(name="weights", bufs=1))
    xpool = ctx.enter_context(tc.tile_pool(name="xchunk", bufs=3))
    xtpool = ctx.enter_context(tc.tile_pool(name="xt", bufs=2))
    qkpool = ctx.enter_context(tc.tile_pool(name="qkt", bufs=2))
    vpool = ctx.enter_context(tc.tile_pool(name="vnat", bufs=2))
    epool = ctx.enter_context(tc.tile_pool(name="exps", bufs=3))
    aopool = ctx.enter_context(tc.tile_pool(name="aot", bufs=2))
    ypool = ctx.enter_context(tc.tile_pool(name="ychunk", bufs=3))
    spool = ctx.enter_context(tc.tile_pool(name="norm", bufs=2))
    aoupool = ctx.enter_context(tc.tile_pool(name="aou", bufs=2))
    ps = ctx.enter_context(tc.tile_pool(name="ps", bufs=4, space="PSUM"))

    def ps_tile(name):
        return ps.tile([P, 2, 512], f32, tag="ps", name=name)

    # ---- constants / weights (resident) ----
    ident = const_pool.tile([P, P], f32, name="ident")
    make_identity(nc, ident)

    ones_f32 = const_pool.tile([1, P], f32, name="ones_f32")
    nc.gpsimd.memset(ones_f32[:], 1.0)
    ones_row = const_pool.tile([1, P], f32r, name="ones_row")
    nc.vector.tensor_copy(ones_row[:], ones_f32[:])

    sel = const_pool.tile([12, CCH, P], bf16, name="sel")
    nc.sync.dma_start(sel[:], sel_ext[:])

    wq_sb = wpool.tile([P, CCH, 3 * C], bf16, name="wq_sb")
    nc.sync.dma_start(wq_sb[:], wq_ext.rearrange("(co p) o -> p co o", p=P))
    pw_sb = wpool.tile([P, CCH, C], bf16, name="pw_sb")
    nc.sync.dma_start(pw_sb[:], pw_ext.rearrange("(co p) o -> p co o", p=P))
    pb_sb = const_pool.tile([1, C], f32r, name="pb_sb")
    nc.sync.dma_start(pb_sb[:], pb_ext[None, :])

    # bias broadcast across partitions: [128, 768] = ones[128,1] @ pb[1,768]
    bias_sb = wpool.tile([P, C], f32, name="bias_sb")
    psb0 = ps_tile("ps_bias")
    for j in range(2):
        nc.tensor.matmul(
            psb0[:, j, 0:384],
            lhsT=ones_row[0:1, :],
            rhs=pb_sb[0:1, j * 384:(j + 1) * 384],
            start=True, stop=True,
        )
        nc.vector.tensor_copy(bias_sb[:, j * 384:(j + 1) * 384], psb0[:, j, 0:384])

    # ---- per-item pipeline ----
    for it in range(ITEMS):
        small = it >= ITEMS // 2
        Heff = H // 2 if small else H
        # o-chunk ids within the q|k layout of qkT (q: 0..5, k: 6..11)
        qk_chunks = ([0, 1, 2, 6, 7, 8] if small else list(range(12)))
        CCH_ao = CCH // 2 if small else CCH   # proj contraction chunks
        NJ = 1 if small else 2                # 384-wide column groups

        # Phase A: load x chunks, transpose to xT [c-part, n-free]
        xT = xtpool.tile([P, CCH, 640], bf16, name="xT")
        nc.gpsimd.memset(xT[:, :, N], 0.0)
        for nch in range(NCH):
            rows = _rows(nch)
            xc = xpool.tile([P, C], f32, name="xc")
            nc.sync.dma_start(xc[0:rows, :], x_ext[it, nch * P:nch * P + rows, :])
            for cc0 in range(0, CCH, 2):
                pst = ps_tile("ps_t")
                for j in (0, 1):
                    cc = cc0 + j
                    nc.tensor.transpose(
                        pst[:, j, 0:rows],
                        xc[0:rows, cc * P:(cc + 1) * P],
                        ident[0:rows, 0:rows],
                    )
                nc.vector.tensor_copy(
                    xT[:, cc0:cc0 + 2, nch * P:nch * P + rows],
                    pst[:, :, 0:rows],
                )

        # Phase B: qT / kT (transposed outputs) for needed o-chunks
        qkT = qkpool.tile([P, 12, N + 1], bf16, name="qkT")
        for oc in qk_chunks:
            wcol = oc * P if oc < 6 else C + (oc - 6) * P
            pqk = ps_tile("ps_qk")
            for j, (n0, nsz) in enumerate(HALVES):
                for cc in range(CCH):
                    nc.tensor.matmul(
                        pqk[:, j, 0:nsz],
                        lhsT=wq_sb[:, cc, wcol:wcol + P],
                        rhs=xT[:, cc, n0:n0 + nsz],
                        start=(cc == 0), stop=(cc == CCH - 1),
                    )
                nc.vector.tensor_copy(qkT[:, oc, n0:n0 + nsz], pqk[:, j, 0:nsz])

        # Phase C: v natural [n-part, (h,d)-free] in bf16 with ones column
        v_nat = vpool.tile([P, NCH, H, D + 2], bf16, name="v_nat")
        nc.gpsimd.memset(v_nat[:, :, :, D:D + 2], 1.0)
        for nch in range(NCH):
            rows = _rows(nch)
            pv = ps_tile("ps_v")
            for j in range(NJ):
                vcol = 2 * C + j * 384
                for cc in range(CCH):
                    nc.tensor.matmul(
                        pv[0:_mcols(nch), j, 0:384],
                        lhsT=xT[:, cc, nch * P:nch * P + _mcols(nch)],
                        rhs=wq_sb[:, cc, vcol:vcol + 384],
                        start=(cc == 0), stop=(cc == CCH - 1),
                    )
                nc.vector.tensor_copy(
                    v_nat[0:rows, nch, j * 6:(j + 1) * 6, 0:D],
                    pv[0:rows, j, 0:384].rearrange("p (h d) -> p h d", h=6),
                )

        # Phase D/E: scores^T (head-pairs packed via tile_position) -> exp
        # -> AV with ones-augmented v (row 64 = softmax denominators).
        # Denominator rows collect in dsum; one batched reciprocal per item,
        # then K=1 broadcast matmuls + DVE multiply normalize into aoT.
        aoT = aopool.tile([P, CCH, N + 1], bf16, name="aoT")
        aoU = aoupool.tile([P, CCH, 2, F0], f32, name="aoU")
        dsum = spool.tile([12, 2, F0], f32, name="dsum")
        for hp in range(Heff // 2):
            kch = 6 + hp
            pse = [None, None]
            expS = [None, None]
            for hh in (0, 1):
                expS[hh] = epool.tile([P, NCH, 2, F0], bf16, tag="expS",
                                      name=f"expS{hh}")
            for mch in range(NCH):
                mrows = _rows(mch)
                for hh in (0, 1):
                    hrow = hh * D
                    pse[hh] = ps_tile(f"ps_s{hh}")
                    for j, (n0, nsz) in enumerate(HALVES):
                        nc.tensor.matmul(
                            pse[hh][0:_mcols(mch), j, 0:nsz],
                            lhsT=qkT[hrow:hrow + D, kch, mch * P:mch * P + _mcols(mch)],
                            rhs=qkT[hrow:hrow + D, hp, n0:n0 + nsz],
                            start=True, stop=True,
                            tile_position=(hrow, 0),
                        )
                for hh in (0, 1):
                    nc.scalar.activation(
                        expS[hh][0:mrows, mch, :, :],
                        pse[hh][0:mrows, :, 0:F0],
                        mybir.ActivationFunctionType.Exp,
                    )
            for hh in (0, 1):
                h = 2 * hp + hh
                po = ps_tile("ps_o")
                for j, (n0, nsz) in enumerate(HALVES):
                    for mch in range(NCH):
                        mrows = _rows(mch)
                        nc.tensor.matmul(
                            po[0:D + 2, j, 0:nsz],
                            lhsT=v_nat[0:mrows, mch, h, :],  # M=66
                            rhs=expS[hh][0:mrows, mch, j, 0:nsz],
                            start=(mch == 0), stop=(mch == NCH - 1),
                        )
                arow = hh * D
                nc.vector.tensor_copy(aoU[arow:arow + D, hp, :, :],
                                      po[0:D, :, 0:F0])
                dstage = spool.tile([1, 2, F0], f32, name="dstage")
                nc.vector.tensor_copy(dstage[0:1, :, :], po[D:D + 1, :, 0:F0])
                nc.sync.dma_start(dsum[h:h + 1, :, :], dstage[0:1, :, :])

        drecip = spool.tile([12, 2, F0], bf16, name="drecip")
        with nc.allow_low_precision(reason="softmax recip bcast via bf16 matmul"):
            nc.vector.reciprocal(drecip[0:Heff, :, :], dsum[0:Heff, :, :])
        for c in range(Heff // 2):
            pbc = ps_tile("ps_bc")
            for j, (n0, nsz) in enumerate(HALVES):
                nc.tensor.matmul(
                    pbc[:, j, 0:nsz],
                    lhsT=sel[0:12, c, :],
                    rhs=drecip[0:12, j, 0:nsz],
                    start=True, stop=True,
                )
            for j, (n0, nsz) in enumerate(HALVES):
                nc.vector.tensor_mul(
                    aoT[:, c, n0:n0 + nsz],
                    aoU[:, c, j, 0:nsz],
                    pbc[:, j, 0:nsz],
                )

        # Phase F: projection + bias (+ zero tail channels for small), DMA out
        for nch in range(NCH):
            rows = _rows(nch)
            psy = ps_tile("ps_y")
            yc = ypool.tile([P, C], f32, name="yc")
            if small:
                nc.gpsimd.memset(yc[0:rows, 384:768], 0.0)
            for j in range(NJ):
                o0 = j * 384
                for cc in range(CCH_ao):
                    nc.tensor.matmul(
                        psy[0:_mcols(nch), j, 0:384],
                        lhsT=aoT[:, cc, nch * P:nch * P + _mcols(nch)],
                        rhs=pw_sb[:, cc, o0:o0 + 384],
                        start=(cc == 0), stop=(cc == CCH_ao - 1),
                    )
                nc.vector.tensor_add(
                    yc[0:rows, o0:o0 + 384],
                    psy[0:rows, j, 0:384],
                    bias_sb[0:rows, o0:o0 + 384],
                )
            nc.sync.dma_start(out_ext[it, nch * P:nch * P + rows, :], yc[0:rows, :])


_GRAPH = None


def _get_graph():
    global _GRAPH
    if _GRAPH is None:
        nc = bacc.Bacc("TRN2", target_bir_lowering=False, debug=False,
                       num_devices=NCORES)
        x_ext = nc.dram_tensor("x", [ITEMS, N, C], f32, kind="ExternalInput").ap()
        wq_ext = nc.dram_tensor("wq", [C, 3 * C], bf16, kind="ExternalInput").ap()
        pw_ext = nc.dram_tensor("pw", [C, C], bf16, kind="ExternalInput").ap()
        pb_ext = nc.dram_tensor("pb", [C], f32r, kind="ExternalInput").ap()
        sel_ext = nc.dram_tensor("sel", [12, CCH, P], bf16, kind="ExternalInput").ap()
        out_ext = nc.dram_tensor("out", [ITEMS, N, C], f32, kind="ExternalOutput").ap()
        with tile.TileContext(nc) as tc:
            with ExitStack() as ctx:
                _emit(ctx, tc, x_ext, wq_ext, pw_ext, pb_ext, sel_ext, out_ext)
        nc.finalize()
        _GRAPH = nc
    return _GRAPH


LAST_RESULTS = None


def kernel(x, qkv_w, proj_w, proj_b, _trace=False):
    global LAST_RESULTS
    x = np.asarray(x, dtype=np.float32)
    wq = np.array(qkv_w, dtype=np.float32)          # copy; rows 0:C are q
    wq[0:C] *= D ** -0.5                            # fold attention scale into Wq
    wqT = np.ascontiguousarray(wq.T).astype(ml_dtypes.bfloat16)   # [C, 3C]
    pwT = np.ascontiguousarray(
        np.asarray(proj_w, dtype=np.float32).T).astype(ml_dtypes.bfloat16)
    pb = np.ascontiguousarray(np.asarray(proj_b, dtype=np.float32))
    sel_np = np.zeros((12, CCH, P), dtype=ml_dtypes.bfloat16)
    for c in range(CCH):
        sel_np[2 * c, c, 0:D] = 1
        sel_np[2 * c + 1, c, D:P] = 1

    nc = _get_graph()
    in_maps = []
    half = x.shape[0] // 2  # 32
    per = half // NCORES    # 4
    for c in range(NCORES):
        xs = np.concatenate(
            [x[per * c:per * (c + 1)], x[half + per * c:half + per * (c + 1)]],
            axis=0,
        )
        in_maps.append({
            "x": np.ascontiguousarray(xs),
            "wq": wqT,
            "pw": pwT,
            "pb": pb,
            "sel": sel_np,
        })

    res = run_bass_kernel_spmd(nc, in_maps, core_ids=list(range(NCORES)),
                               trace=_trace)
    LAST_RESULTS = res

    out = np.empty((x.shape[0], N, C), dtype=np.float32)
    for c in range(NCORES):
        o = res.results[c]["out"]
        out[per * c:per * (c + 1)] = o[0:per]
        out[half + per * c:half + per * (c + 1)] = o[per:2 * per]
    return out


# revision 15
# speedup vs baseline: 1.0472x; 1.0343x over previous
"""Sparse multi-head attention (ViT-style, 577 tokens, 12 heads) on 8 TRN2
NeuronCores.

Sharding: pure data-parallel over batch. Each core gets 8 of the 64 batch
items: 4 from the "large" half (full 12-head attention) and 4 from the
"small" half (compressed: heads 6..11 of q/k/v are statically zero, so only
6 heads + a 384x384 projection are computed). Co-sharding large/small
halves balances per-core compute. No collectives are needed.

Per-item dataflow (everything stays in the transposed domain so no
intermediate ever needs a device transpose except the initial x -> xT):

  x[577,768] --PE-transpose--> xT[c,n]
  qT,kT[o,n] = Wqkv^T-stationary matmuls over xT     (q pre-scaled by D^-0.5)
  v[n,o]     = xT-stationary matmuls over Wv^T, plus a ones column (aug)
  S^T[m,n]   = kT-stationary over qT (per head, K=64)
  P^T        = exp(S^T)  (scalar engine, PSUM->SBUF, bf16; softmax max-shift
               skipped: logits are O(1) by construction)
  aoT[d,n]   = v_aug^T @ P^T  -> row 64 holds the softmax denominators
  normalize via reciprocal + K=1 broadcast matmul
  y[n,oc]    = aoT-stationary over proj_w^T, + bias, DMA out.

Matmuls run as float32r (full PE rate at free-dim >= 256) on fp32 data;
only P^T and v are bf16 (flash-attention-style precision).
"""

import ml_dtypes
import numpy as np
from contextlib import ExitStack

import concourse.bass as bass
import concourse.tile as tile
from concourse import bacc, mybir
from concourse import bass2jax as _b2j
from concourse.bass_utils import run_bass_kernel_spmd
from concourse.masks import make_identity


def _run_bass_via_pjrt_presharded(nc, in_maps, n_cores):
    """Drop-in replacement for bass2jax.run_bass_via_pjrt (multi-core path).

    The stock version concatenates per-core inputs into one host array and
    lets jax reshard it onto the mesh; on the neuron PJRT backend that
    resharding lowers to a compiled "scatter" program which, for ~100MB
    inputs, dies in neuronx-cc codegen (16-bit semaphore_wait_value
    overflow). Here each per-core shard is device_put directly onto its
    device and the global array is assembled zero-copy, so the jitted body
    sees correctly-sharded operands and no data-movement program exists.
    """
    import jax

    _b2j.install_neuronx_cc_hook()
    assert nc.dbg_addr is None and nc.partition_id_tensor is None

    from jax.experimental.shard_map import shard_map
    from jax.sharding import Mesh, NamedSharding, PartitionSpec

    in_names, out_names, out_avals, zero_shapes = [], [], [], []
    for alloc in nc.m.functions[0].allocations:
        if not isinstance(alloc, mybir.MemoryLocationSet):
            continue
        name = alloc.memorylocations[0].name
        if alloc.kind == "ExternalInput":
            in_names.append(name)
        elif alloc.kind == "ExternalOutput":
            shape = tuple(alloc.tensor_shape)
            dtype = mybir.dt.np(alloc.dtype)
            out_names.append(name)
            out_avals.append(jax.core.ShapedArray(shape, dtype))
            zero_shapes.append((shape, dtype))
    n_params = len(in_names)
    n_outs = len(out_names)
    all_names = in_names + out_names
    donate = tuple(range(n_params, n_params + n_outs))

    def _body(*args):
        outs = _b2j._bass_exec_p.bind(
            *args,
            out_avals=tuple(out_avals),
            in_names=tuple(all_names),
            out_names=tuple(out_names),
            lowering_input_output_aliases=(),
            sim_require_finite=True,
            sim_require_nnan=True,
            nc=nc,
        )
        return tuple(outs)

    devices = jax.devices()[:n_cores]
    mesh = Mesh(np.asarray(devices), ("core",))
    sharding = NamedSharding(mesh, PartitionSpec("core"))

    def make_global(shards):
        s0 = np.asarray(shards[0])
        gshape = (n_cores * s0.shape[0], *s0.shape[1:])
        parts = [
            jax.device_put(np.ascontiguousarray(shards[c]), devices[c])
            for c in range(n_cores)
        ]
        return jax.make_array_from_single_device_arrays(gshape, sharding, parts)

    global_ins = [make_global([m[nm] for m in in_maps]) for nm in in_names]
    global_zeros = [
        make_global([np.zeros(shape, dtype)] * n_cores)
        for shape, dtype in zero_shapes
    ]

    sharded = jax.jit(
        shard_map(_body, mesh=mesh, in_specs=(PartitionSpec("core"),) * (n_params + n_outs),
                  out_specs=(PartitionSpec("core"),) * n_outs, check_rep=False),
        donate_argnums=donate,
        keep_unused=True,
    )
    out_arrs = sharded(*global_ins, *global_zeros)

    results = [dict() for _ in range(n_cores)]
    for i, name in enumerate(out_names):
        arr = out_arrs[i]
        per = {s.index[0].start or 0: np.asarray(s.data) for s in arr.addressable_shards}
        step = out_avals[i].shape[0]
        for c in range(n_cores):
            results[c][name] = per[c * step]
    return results


def _patched_run_bass_via_pjrt(nc, in_maps, n_cores):
    if n_cores > 1 and nc.partition_id_tensor is None and nc.dbg_addr is None:
        return _run_bass_via_pjrt_presharded(nc, in_maps, n_cores)
    return _orig_run_bass_via_pjrt(nc, in_maps, n_cores)


_orig_run_bass_via_pjrt = _b2j.run_bass_via_pjrt
_b2j.run_bass_via_pjrt = _patched_run_bass_via_pjrt

P = 128
N = 577
C = 768
H = 12
D = 64
NCH = 5           # n (token) chunks: 4*128 + 65
CCH = 6           # c chunks: 768 / 128
NTAIL = N - 4 * P  # 65
F0, F1 = 290, 288  # n free-dim halves, padded n=578: fp32r needs EVEN free sizes
HALVES = ((0, F0), (F0, F1))
ITEMS = 8
NCORES = 8

f32 = mybir.dt.float32
f32r = mybir.dt.float32r
bf16 = mybir.dt.bfloat16


def _rows(nch):
    return NTAIL if nch == NCH - 1 else P


def _mcols(nch):
    """lhsT column count for an n-chunk: pad the 65-tail to 66 (even M is
    measurably faster on the PE); the extra output partition is discarded."""
    return NTAIL + 1 if nch == NCH - 1 else P


def _emit(ctx, tc, x_ext, wq_ext, pw_ext, pb_ext, sel_ext, out_ext):
    nc = tc.nc

    const_pool = ctx.enter_context(tc.tile_pool(name="const", bufs=1))
    wpool = ctx.enter_context(tc.tile_pool(name="weights", bufs=1))
    xpool = ctx.enter_context(tc.tile_pool(name="xchunk", bufs=3))
    xtpool = ctx.enter_context(tc.tile_pool(name="xt", bufs=2))
    qkpool = ctx.enter_context(tc.tile_pool(name="qkt", bufs=2))
    vpool = ctx.enter_context(tc.tile_pool(name="vnat", bufs=2))
    epool = ctx.enter_context(tc.tile_pool(name="exps", bufs=3))
    aopool = ctx.enter_context(tc.tile_pool(name="aot", bufs=2))
    ypool = ctx.enter_context(tc.tile_pool(name="ychunk", bufs=3))
    spool = ctx.enter_context(tc.tile_pool(name="norm", bufs=2))
    aoupool = ctx.enter_context(tc.tile_pool(name="aou", bufs=2))
    ps = ctx.enter_context(tc.tile_pool(name="ps", bufs=4, space="PSUM"))

    def ps_tile(name):
        return ps.tile([P, 2, 512], f32, tag="ps", name=name)

    # ---- constants / weights (resident) ----
    ident = const_pool.tile([P, P], f32, name="ident")
    make_identity(nc, ident)

    ones_f32 = const_pool.tile([1, P], f32, name="ones_f32")
    nc.gpsimd.memset(ones_f32[:], 1.0)
    ones_row = const_pool.tile([1, P], f32r, name="ones_row")
    nc.vector.tensor_copy(ones_row[:], ones_f32[:])

    sel = const_pool.tile([12, CCH, P], bf16, name="sel")
    nc.sync.dma_start(sel[:], sel_ext[:])

    wq_sb = wpool.tile([P, CCH, 3 * C], bf16, name="wq_sb")
    nc.sync.dma_start(wq_sb[:], wq_ext.rearrange("(co p) o -> p co o", p=P))
    pw_sb = wpool.tile([P, CCH, C], bf16, name="pw_sb")
    nc.sync.dma_start(pw_sb[:], pw_ext.rearrange("(co p) o -> p co o", p=P))
    pb_sb = const_pool.tile([1, C], f32r, name="pb_sb")
    nc.sync.dma_start(pb_sb[:], pb_ext[None, :])

    # bias broadcast across partitions: [128, 768] = ones[128,1] @ pb[1,768]
    bias_sb = wpool.tile([P, C], f32, name="bias_sb")
    psb0 = ps_tile("ps_bias")
    for j in range(2):
        nc.tensor.matmul(
            psb0[:, j, 0:384],
            lhsT=ones_row[0:1, :],
            rhs=pb_sb[0:1, j * 384:(j + 1) * 384],
            start=True, stop=True,
        )
        nc.vector.tensor_copy(bias_sb[:, j * 384:(j + 1) * 384], psb0[:, j, 0:384])

    # ---- per-item pipeline (1-item software pipeline) ----
    # The normalization tail of item i (recip bcast + multiply + projection
    # + output DMA) is emitted AFTER item i+1's qkv phase: the PE executes
    # its stream in order, so this gives the denominator-staging DMAs and
    # the batched reciprocal of item i ~20us of unrelated PE work to hide
    # behind instead of stalling the PE at the broadcast matmuls.

    def emit_tail(st):
        aoT, aoU, drecip, it, small = st
        Heff = H // 2 if small else H
        CCH_ao = CCH // 2 if small else CCH
        NJ = 1 if small else 2
        for c in range(Heff // 2):
            pbc = ps_tile("ps_bc")
            for j, (n0, nsz) in enumerate(HALVES):
                nc.tensor.matmul(
                    pbc[:, j, 0:nsz],
                    lhsT=sel[0:12, c, :],
                    rhs=drecip[0:12, j, 0:nsz],
                    start=True, stop=True,
                )
            for j, (n0, nsz) in enumerate(HALVES):
                nc.vector.tensor_mul(
                    aoT[:, c, n0:n0 + nsz],
                    aoU[:, c, j, 0:nsz],
                    pbc[:, j, 0:nsz],
                )
        # projection + bias (+ zero tail channels for small), DMA out
        for nch in range(NCH):
            rows = _rows(nch)
            psy = ps_tile("ps_y")
            yc = ypool.tile([P, C], f32, name="yc")
            if small:
                nc.gpsimd.memset(yc[0:rows, 384:768], 0.0)
            for j in range(NJ):
                o0 = j * 384
                for cc in range(CCH_ao):
                    nc.tensor.matmul(
                        psy[0:_mcols(nch), j, 0:384],
                        lhsT=aoT[:, cc, nch * P:nch * P + _mcols(nch)],
                        rhs=pw_sb[:, cc, o0:o0 + 384],
                        start=(cc == 0), stop=(cc == CCH_ao - 1),
                    )
                nc.vector.tensor_add(
                    yc[0:rows, o0:o0 + 384],
                    psy[0:rows, j, 0:384],
                    bias_sb[0:rows, o0:o0 + 384],
                )
            nc.gpsimd.dma_start(out_ext[it, nch * P:nch * P + rows, :],
                                yc[0:rows, :])

    pending = None
    for it in range(ITEMS):
        small = it >= ITEMS // 2
        Heff = H // 2 if small else H
        # o-chunk ids within the q|k layout of qkT (q: 0..5, k: 6..11)
        qk_chunks = ([0, 1, 2, 6, 7, 8] if small else list(range(12)))
        NJ = 1 if small else 2                # 384-wide column groups

        # Phase A: load x chunks, transpose to xT [c-part, n-free]
        xT = xtpool.tile([P, CCH, 640], bf16, name="xT")
        nc.gpsimd.memset(xT[:, :, N], 0.0)
        for nch in range(NCH):
            rows = _rows(nch)
            xc = xpool.tile([P, C], f32, name="xc")
            nc.sync.dma_start(xc[0:rows, :], x_ext[it, nch * P:nch * P + rows, :])
            for cc0 in range(0, CCH, 2):
                pst = ps_tile("ps_t")
                for j in (0, 1):
                    cc = cc0 + j
                    nc.tensor.transpose(
                        pst[:, j, 0:rows],
                        xc[0:rows, cc * P:(cc + 1) * P],
                        ident[0:rows, 0:rows],
                    )
                nc.vector.tensor_copy(
                    xT[:, cc0:cc0 + 2, nch * P:nch * P + rows],
                    pst[:, :, 0:rows],
                )

        # Phase B: qT / kT (transposed outputs) for needed o-chunks
        qkT = qkpool.tile([P, 12, N + 1], bf16, name="qkT")
        for oc in qk_chunks:
            wcol = oc * P if oc < 6 else C + (oc - 6) * P
            pqk = ps_tile("ps_qk")
            for j, (n0, nsz) in enumerate(HALVES):
                for cc in range(CCH):
                    nc.tensor.matmul(
                        pqk[:, j, 0:nsz],
                        lhsT=wq_sb[:, cc, wcol:wcol + P],
                        rhs=xT[:, cc, n0:n0 + nsz],
                        start=(cc == 0), stop=(cc == CCH - 1),
                    )
                nc.vector.tensor_copy(qkT[:, oc, n0:n0 + nsz], pqk[:, j, 0:nsz])

        # deferred tail of the previous item, hidden behind this item's qkv
        if pending is not None:
            emit_tail(pending)
            pending = None

        # Phase C: v natural [n-part, (h,d)-free] in bf16 with ones column
        v_nat = vpool.tile([P, NCH, H, D + 2], bf16, name="v_nat")
        nc.gpsimd.memset(v_nat[:, :, :, D:D + 2], 1.0)
        for nch in range(NCH):
            rows = _rows(nch)
            pv = ps_tile("ps_v")
            for j in range(NJ):
                vcol = 2 * C + j * 384
                for cc in range(CCH):
                    nc.tensor.matmul(
                        pv[0:_mcols(nch), j, 0:384],
                        lhsT=xT[:, cc, nch * P:nch * P + _mcols(nch)],
                        rhs=wq_sb[:, cc, vcol:vcol + 384],
                        start=(cc == 0), stop=(cc == CCH - 1),
                    )
                nc.vector.tensor_copy(
                    v_nat[0:rows, nch, j * 6:(j + 1) * 6, 0:D],
                    pv[0:rows, j, 0:384].rearrange("p (h d) -> p h d", h=6),
                )

        # Phase D/E: scores^T (head-pairs packed via tile_position) -> exp
        # -> AV with ones-augmented v (row 64 = softmax denominators).
        aoT = aopool.tile([P, CCH, N + 1], bf16, name="aoT")
        aoU = aoupool.tile([P, CCH, 2, F0], f32, name="aoU")
        dsum = spool.tile([12, 2, F0], f32, name="dsum")
        for hp in range(Heff // 2):
            kch = 6 + hp
            pse = [None, None]
            expS = [None, None]
            for hh in (0, 1):
                expS[hh] = epool.tile([P, NCH, 2, F0], bf16, tag="expS",
                                      name=f"expS{hh}")
            for mch in range(NCH):
                mrows = _rows(mch)
                for hh in (0, 1):
                    hrow = hh * D
                    pse[hh] = ps_tile(f"ps_s{hh}")
                    for j, (n0, nsz) in enumerate(HALVES):
                        nc.tensor.matmul(
                            pse[hh][0:_mcols(mch), j, 0:nsz],
                            lhsT=qkT[hrow:hrow + D, kch, mch * P:mch * P + _mcols(mch)],
                            rhs=qkT[hrow:hrow + D, hp, n0:n0 + nsz],
                            start=True, stop=True,
                            tile_position=(hrow, 0),
                        )
                for hh in (0, 1):
                    nc.scalar.activation(
                        expS[hh][0:mrows, mch, :, :],
                        pse[hh][0:mrows, :, 0:F0],
                        mybir.ActivationFunctionType.Exp,
                    )
            for hh in (0, 1):
                h = 2 * hp + hh
                po = ps_tile("ps_o")
                for j, (n0, nsz) in enumerate(HALVES):
                    for mch in range(NCH):
                        mrows = _rows(mch)
                        nc.tensor.matmul(
                            po[0:D + 2, j, 0:nsz],
                            lhsT=v_nat[0:mrows, mch, h, :],  # M=66
                            rhs=expS[hh][0:mrows, mch, j, 0:nsz],
                            start=(mch == 0), stop=(mch == NCH - 1),
                        )
                arow = hh * D
                nc.vector.tensor_copy(aoU[arow:arow + D, hp, :, :],
                                      po[0:D, :, 0:F0])
                dstage = spool.tile([1, 2, F0], f32, name="dstage")
                nc.vector.tensor_copy(dstage[0:1, :, :], po[D:D + 1, :, 0:F0])
                nc.gpsimd.dma_start(dsum[h:h + 1, :, :], dstage[0:1, :, :])

        drecip = spool.tile([12, 2, F0], bf16, name="drecip")
        with nc.allow_low_precision(reason="softmax recip bcast via bf16 matmul"):
            nc.vector.reciprocal(drecip[0:Heff, :, :], dsum[0:Heff, :, :])
        pending = (aoT, aoU, drecip, it, small)

    emit_tail(pending)


_GRAPH = None


def _get_graph():
    global _GRAPH
    if _GRAPH is None:
        nc = bacc.Bacc("TRN2", target_bir_lowering=False, debug=False,
                       num_devices=NCORES)
        x_ext = nc.dram_tensor("x", [ITEMS, N, C], f32, kind="ExternalInput").ap()
        wq_ext = nc.dram_tensor("wq", [C, 3 * C], bf16, kind="ExternalInput").ap()
        pw_ext = nc.dram_tensor("pw", [C, C], bf16, kind="ExternalInput").ap()
        pb_ext = nc.dram_tensor("pb", [C], f32r, kind="ExternalInput").ap()
        sel_ext = nc.dram_tensor("sel", [12, CCH, P], bf16, kind="ExternalInput").ap()
        out_ext = nc.dram_tensor("out", [ITEMS, N, C], f32, kind="ExternalOutput").ap()
        with tile.TileContext(nc) as tc:
            with ExitStack() as ctx:
                _emit(ctx, tc, x_ext, wq_ext, pw_ext, pb_ext, sel_ext, out_ext)
        nc.finalize()
        _GRAPH = nc
    return _GRAPH


LAST_RESULTS = None


def kernel(x, qkv_w, proj_w, proj_b, _trace=False):
    global LAST_RESULTS
    x = np.asarray(x, dtype=np.float32)
    wq = np.array(qkv_w, dtype=np.float32)          # copy; rows 0:C are q
    wq[0:C] *= D ** -0.5                            # fold attention scale into Wq
    wqT = np.ascontiguousarray(wq.T).astype(ml_dtypes.bfloat16)   # [C, 3C]
    pwT = np.ascontiguousarray(
        np.asarray(proj_w, dtype=np.float32).T).astype(ml_dtypes.bfloat16)
    pb = np.ascontiguousarray(np.asarray(proj_b, dtype=np.float32))
    sel_np = np.zeros((12, CCH, P), dtype=ml_dtypes.bfloat16)
    for c in range(CCH):
        sel_np[2 * c, c, 0:D] = 1
        sel_np[2 * c + 1, c, D:P] = 1

    nc = _get_graph()
    in_maps = []
    half = x.shape[0] // 2  # 32
    per = half // NCORES    # 4
    for c in range(NCORES):
        xs = np.concatenate(
            [x[per * c:per * (c + 1)], x[half + per * c:half + per * (c + 1)]],
            axis=0,
        )
        in_maps.append({
            "x": np.ascontiguousarray(xs),
            "wq": wqT,
            "pw": pwT,
            "pb": pb,
            "sel": sel_np,
        })

    res = run_bass_kernel_spmd(nc, in_maps, core_ids=list(range(NCORES)),
                               trace=_trace)
    LAST_RESULTS = res

    out = np.empty((x.shape[0], N, C), dtype=np.float32)
    for c in range(NCORES):
        o = res.results[c]["out"]
        out[per * c:per * (c + 1)] = o[0:per]
        out[half + per * c:half + per * (c + 1)] = o[per:2 * per]
    return out


# revision 17
# speedup vs baseline: 1.0847x; 1.0359x over previous
"""Sparse multi-head attention (ViT-style, 577 tokens, 12 heads) on 8 TRN2
NeuronCores.

Sharding: pure data-parallel over batch. Each core gets 8 of the 64 batch
items: 4 from the "large" half (full 12-head attention) and 4 from the
"small" half (compressed: heads 6..11 of q/k/v are statically zero, so only
6 heads + a 384x384 projection are computed). Co-sharding large/small
halves balances per-core compute. No collectives are needed.

Per-item dataflow (everything stays in the transposed domain so no
intermediate ever needs a device transpose except the initial x -> xT):

  x[577,768] --PE-transpose--> xT[c,n]
  qT,kT[o,n] = Wqkv^T-stationary matmuls over xT     (q pre-scaled by D^-0.5)
  v[n,o]     = xT-stationary matmuls over Wv^T, plus a ones column (aug)
  S^T[m,n]   = kT-stationary over qT (per head, K=64)
  P^T        = exp(S^T)  (scalar engine, PSUM->SBUF, bf16; softmax max-shift
               skipped: logits are O(1) by construction)
  aoT[d,n]   = v_aug^T @ P^T  -> row 64 holds the softmax denominators
  normalize via reciprocal + K=1 broadcast matmul
  y[n,oc]    = aoT-stationary over proj_w^T, + bias, DMA out.

Matmuls run as float32r (full PE rate at free-dim >= 256) on fp32 data;
only P^T and v are bf16 (flash-attention-style precision).
"""

import ml_dtypes
import numpy as np
from contextlib import ExitStack

import concourse.bass as bass
import concourse.tile as tile
from concourse import bacc, mybir
from concourse import bass2jax as _b2j
from concourse.bass_utils import run_bass_kernel_spmd
from concourse.masks import make_identity


def _run_bass_via_pjrt_presharded(nc, in_maps, n_cores):
    """Drop-in replacement for bass2jax.run_bass_via_pjrt (multi-core path).

    The stock version concatenates per-core inputs into one host array and
    lets jax reshard it onto the mesh; on the neuron PJRT backend that
    resharding lowers to a compiled "scatter" program which, for ~100MB
    inputs, dies in neuronx-cc codegen (16-bit semaphore_wait_value
    overflow). Here each per-core shard is device_put directly onto its
    device and the global array is assembled zero-copy, so the jitted body
    sees correctly-sharded operands and no data-movement program exists.
    """
    import jax

    _b2j.install_neuronx_cc_hook()
    assert nc.dbg_addr is None and nc.partition_id_tensor is None

    from jax.experimental.shard_map import shard_map
    from jax.sharding import Mesh, NamedSharding, PartitionSpec

    in_names, out_names, out_avals, zero_shapes = [], [], [], []
    for alloc in nc.m.functions[0].allocations:
        if not isinstance(alloc, mybir.MemoryLocationSet):
            continue
        name = alloc.memorylocations[0].name
        if alloc.kind == "ExternalInput":
            in_names.append(name)
        elif alloc.kind == "ExternalOutput":
            shape = tuple(alloc.tensor_shape)
            dtype = mybir.dt.np(alloc.dtype)
            out_names.append(name)
            out_avals.append(jax.core.ShapedArray(shape, dtype))
            zero_shapes.append((shape, dtype))
    n_params = len(in_names)
    n_outs = len(out_names)
    all_names = in_names + out_names
    donate = tuple(range(n_params, n_params + n_outs))

    def _body(*args):
        outs = _b2j._bass_exec_p.bind(
            *args,
            out_avals=tuple(out_avals),
            in_names=tuple(all_names),
            out_names=tuple(out_names),
            lowering_input_output_aliases=(),
            sim_require_finite=True,
            sim_require_nnan=True,
            nc=nc,
        )
        return tuple(outs)

    devices = jax.devices()[:n_cores]
    mesh = Mesh(np.asarray(devices), ("core",))
    sharding = NamedSharding(mesh, PartitionSpec("core"))

    def make_global(shards):
        s0 = np.asarray(shards[0])
        gshape = (n_cores * s0.shape[0], *s0.shape[1:])
        parts = [
            jax.device_put(np.ascontiguousarray(shards[c]), devices[c])
            for c in range(n_cores)
        ]
        return jax.make_array_from_single_device_arrays(gshape, sharding, parts)

    global_ins = [make_global([m[nm] for m in in_maps]) for nm in in_names]
    global_zeros = [
        make_global([np.zeros(shape, dtype)] * n_cores)
        for shape, dtype in zero_shapes
    ]

    sharded = jax.jit(
        shard_map(_body, mesh=mesh, in_specs=(PartitionSpec("core"),) * (n_params + n_outs),
                  out_specs=(PartitionSpec("core"),) * n_outs, check_rep=False),
        donate_argnums=donate,
        keep_unused=True,
    )
    out_arrs = sharded(*global_ins, *global_zeros)

    results = [dict() for _ in range(n_cores)]
    for i, name in enumerate(out_names):
        arr = out_arrs[i]
        per = {s.index[0].start or 0: np.asarray(s.data) for s in arr.addressable_shards}
        step = out_avals[i].shape[0]
        for c in range(n_cores):
            results[c][name] = per[c * step]
    return results


def _patched_run_bass_via_pjrt(nc, in_maps, n_cores):
    if n_cores > 1 and nc.partition_id_tensor is None and nc.dbg_addr is None:
        return _run_bass_via_pjrt_presharded(nc, in_maps, n_cores)
    return _orig_run_bass_via_pjrt(nc, in_maps, n_cores)


_orig_run_bass_via_pjrt = _b2j.run_bass_via_pjrt
_b2j.run_bass_via_pjrt = _patched_run_bass_via_pjrt

P = 128
N = 577
C = 768
H = 12
D = 64
NCH = 5           # n (token) chunks: 4*128 + 65
CCH = 6           # c chunks: 768 / 128
NTAIL = N - 4 * P  # 65
F0, F1 = 290, 288  # n free-dim halves, padded n=578: fp32r needs EVEN free sizes
HALVES = ((0, F0), (F0, F1))
ITEMS = 8
NCORES = 8

f32 = mybir.dt.float32
f32r = mybir.dt.float32r
bf16 = mybir.dt.bfloat16


def _rows(nch):
    return NTAIL if nch == NCH - 1 else P


def _mcols(nch):
    """lhsT column count for an n-chunk: pad the 65-tail to 66 (even M is
    measurably faster on the PE); the extra output partition is discarded."""
    return NTAIL + 1 if nch == NCH - 1 else P


def _emit(ctx, tc, x_ext, wq_ext, pw_ext, pb_ext, sel_ext, out_ext):
    nc = tc.nc

    const_pool = ctx.enter_context(tc.tile_pool(name="const", bufs=1))
    wpool = ctx.enter_context(tc.tile_pool(name="weights", bufs=1))
    xpool = ctx.enter_context(tc.tile_pool(name="xchunk", bufs=1))
    xtpool = ctx.enter_context(tc.tile_pool(name="xt", bufs=2))
    qkpool = ctx.enter_context(tc.tile_pool(name="qkt", bufs=2))
    vpool = ctx.enter_context(tc.tile_pool(name="vnat", bufs=2))
    epool = ctx.enter_context(tc.tile_pool(name="exps", bufs=3))
    aopool = ctx.enter_context(tc.tile_pool(name="aot", bufs=2))
    ypool = ctx.enter_context(tc.tile_pool(name="ychunk", bufs=3))
    spool = ctx.enter_context(tc.tile_pool(name="norm", bufs=2))
    aoupool = ctx.enter_context(tc.tile_pool(name="aou", bufs=2))
    ps = ctx.enter_context(tc.tile_pool(name="ps", bufs=4, space="PSUM"))

    def ps_tile(name):
        return ps.tile([P, 2, 512], f32, tag="ps", name=name)

    # ---- constants / weights (resident) ----
    ident = const_pool.tile([P, P], f32, name="ident")
    make_identity(nc, ident)

    ones_f32 = const_pool.tile([1, P], f32, name="ones_f32")
    nc.gpsimd.memset(ones_f32[:], 1.0)
    ones_row = const_pool.tile([1, P], f32r, name="ones_row")
    nc.vector.tensor_copy(ones_row[:], ones_f32[:])

    sel = const_pool.tile([12, CCH, P], bf16, name="sel")
    nc.sync.dma_start(sel[:], sel_ext[:])

    wq_sb = wpool.tile([P, CCH, 3 * C], bf16, name="wq_sb")
    nc.sync.dma_start(wq_sb[:], wq_ext.rearrange("(co p) o -> p co o", p=P))
    pw_sb = wpool.tile([P, CCH, C], bf16, name="pw_sb")
    nc.sync.dma_start(pw_sb[:], pw_ext.rearrange("(co p) o -> p co o", p=P))
    pb_sb = const_pool.tile([1, C], f32r, name="pb_sb")
    nc.sync.dma_start(pb_sb[:], pb_ext[None, :])

    # bias broadcast across partitions: [128, 768] = ones[128,1] @ pb[1,768]
    bias_sb = wpool.tile([P, C], f32, name="bias_sb")
    psb0 = ps_tile("ps_bias")
    for j in range(2):
        nc.tensor.matmul(
            psb0[:, j, 0:384],
            lhsT=ones_row[0:1, :],
            rhs=pb_sb[0:1, j * 384:(j + 1) * 384],
            start=True, stop=True,
        )
        nc.vector.tensor_copy(bias_sb[:, j * 384:(j + 1) * 384], psb0[:, j, 0:384])

    # ---- per-item pipeline (1-item software pipeline) ----
    # The normalization tail of item i (recip bcast + multiply + projection
    # + output DMA) is emitted AFTER item i+1's qkv phase: the PE executes
    # its stream in order, so this gives the denominator-staging DMAs and
    # the batched reciprocal of item i ~20us of unrelated PE work to hide
    # behind instead of stalling the PE at the broadcast matmuls.

    def emit_tail(st):
        aoT, aoU, drecip, it, small = st
        Heff = H // 2 if small else H
        CCH_ao = CCH // 2 if small else CCH
        NJ = 1 if small else 2
        for c in range(Heff // 2):
            pbc = ps_tile("ps_bc")
            for j, (n0, nsz) in enumerate(HALVES):
                nc.tensor.matmul(
                    pbc[:, j, 0:nsz],
                    lhsT=sel[0:12, c, :],
                    rhs=drecip[0:12, j, 0:nsz],
                    start=True, stop=True,
                )
            for j, (n0, nsz) in enumerate(HALVES):
                nc.vector.tensor_mul(
                    aoT[:, c, n0:n0 + nsz],
                    aoU[:, c, j, 0:nsz],
                    pbc[:, j, 0:nsz],
                )
        # projection + bias (+ zero tail channels for small), DMA out
        for nch in range(NCH):
            rows = _rows(nch)
            psy = ps_tile("ps_y")
            yc = ypool.tile([P, C], f32, name="yc")
            if small:
                nc.gpsimd.memset(yc[0:rows, 384:768], 0.0)
            for j in range(NJ):
                o0 = j * 384
                for cc in range(CCH_ao):
                    nc.tensor.matmul(
                        psy[0:_mcols(nch), j, 0:384],
                        lhsT=aoT[:, cc, nch * P:nch * P + _mcols(nch)],
                        rhs=pw_sb[:, cc, o0:o0 + 384],
                        start=(cc == 0), stop=(cc == CCH_ao - 1),
                    )
                nc.vector.tensor_add(
                    yc[0:rows, o0:o0 + 384],
                    psy[0:rows, j, 0:384],
                    bias_sb[0:rows, o0:o0 + 384],
                )
            nc.gpsimd.dma_start(out_ext[it, nch * P:nch * P + rows, :],
                                yc[0:rows, :])

    pending = None
    for it in range(ITEMS):
        small = it >= ITEMS // 2
        Heff = H // 2 if small else H
        # o-chunk ids within the q|k layout of qkT (q: 0..5, k: 6..11)
        qk_chunks = ([0, 1, 2, 6, 7, 8] if small else list(range(12)))
        NJ = 1 if small else 2                # 384-wide column groups

        # Phase A: load x (2 batched DMAs), transpose to xT [c-part, n-free]
        xT = xtpool.tile([P, CCH, 640], bf16, name="xT")
        nc.gpsimd.memset(xT[:, :, N], 0.0)
        xn = xpool.tile([P, NCH, C], f32, name="xn")
        nc.sync.dma_start(
            xn[:, 0:4, :],
            x_ext[it, 0:4 * P, :].rearrange("(o p) c -> p o c", p=P))
        nc.sync.dma_start(xn[0:NTAIL, 4, :], x_ext[it, 4 * P:N, :])
        for nch in range(NCH):
            rows = _rows(nch)
            for cc0 in range(0, CCH, 2):
                pst = ps_tile("ps_t")
                for j in (0, 1):
                    cc = cc0 + j
                    nc.tensor.transpose(
                        pst[:, j, 0:rows],
                        xn[0:rows, nch, cc * P:(cc + 1) * P],
                        ident[0:rows, 0:rows],
                    )
                nc.vector.tensor_copy(
                    xT[:, cc0:cc0 + 2, nch * P:nch * P + rows],
                    pst[:, :, 0:rows],
                )

        # Phase B: qT / kT (transposed outputs) for needed o-chunks
        qkT = qkpool.tile([P, 12, N + 1], bf16, name="qkT")
        for oc in qk_chunks:
            wcol = oc * P if oc < 6 else C + (oc - 6) * P
            pqk = ps_tile("ps_qk")
            for j, (n0, nsz) in enumerate(HALVES):
                for cc in range(CCH):
                    nc.tensor.matmul(
                        pqk[:, j, 0:nsz],
                        lhsT=wq_sb[:, cc, wcol:wcol + P],
                        rhs=xT[:, cc, n0:n0 + nsz],
                        start=(cc == 0), stop=(cc == CCH - 1),
                    )
                nc.vector.tensor_copy(qkT[:, oc, n0:n0 + nsz], pqk[:, j, 0:nsz])

        # deferred tail of the previous item, hidden behind this item's qkv
        if pending is not None:
            emit_tail(pending)
            pending = None

        # Phase C: v natural [n-part, (h,d)-free] in bf16 with ones column
        v_nat = vpool.tile([P, NCH, H, 2 * D], bf16, name="v_nat")
        nc.gpsimd.memset(v_nat[:, :, :, D:2 * D], 0.0)
        nc.gpsimd.memset(v_nat[:, :, :, D], 1.0)
        for nch in range(NCH):
            rows = _rows(nch)
            pv = ps_tile("ps_v")
            for j in range(NJ):
                vcol = 2 * C + j * 384
                for cc in range(CCH):
                    nc.tensor.matmul(
                        pv[0:_mcols(nch), j, 0:384],
                        lhsT=xT[:, cc, nch * P:nch * P + _mcols(nch)],
                        rhs=wq_sb[:, cc, vcol:vcol + 384],
                        start=(cc == 0), stop=(cc == CCH - 1),
                    )
                nc.vector.tensor_copy(
                    v_nat[0:rows, nch, j * 6:(j + 1) * 6, 0:D],
                    pv[0:rows, j, 0:384].rearrange("p (h d) -> p h d", h=6),
                )

        # Phase D/E: scores^T (head-pairs packed via tile_position) -> exp
        # -> AV with ones-augmented v (row 64 = softmax denominators).
        aoT = aopool.tile([P, CCH, N + 1], bf16, name="aoT")
        aoU = aoupool.tile([P, CCH, 2, F0], bf16, name="aoU")
        dsum = spool.tile([12, 2, F0], f32, name="dsum")
        for hp in range(Heff // 2):
            kch = 6 + hp
            pse = [None, None]
            expS = [None, None]
            for hh in (0, 1):
                expS[hh] = epool.tile([P, NCH, 2, F0], bf16, tag="expS",
                                      name=f"expS{hh}")
            for mch in range(NCH):
                mrows = _rows(mch)
                for hh in (0, 1):
                    hrow = hh * D
                    pse[hh] = ps_tile(f"ps_s{hh}")
                    for j, (n0, nsz) in enumerate(HALVES):
                        nc.tensor.matmul(
                            pse[hh][0:_mcols(mch), j, 0:nsz],
                            lhsT=qkT[hrow:hrow + D, kch, mch * P:mch * P + _mcols(mch)],
                            rhs=qkT[hrow:hrow + D, hp, n0:n0 + nsz],
                            start=True, stop=True,
                            tile_position=(hrow, 0),
                        )
                for hh in (0, 1):
                    nc.scalar.activation(
                        expS[hh][0:mrows, mch, :, :],
                        pse[hh][0:mrows, :, 0:F0],
                        mybir.ActivationFunctionType.Exp,
                    )
            for hh in (0, 1):
                h = 2 * hp + hh
                po = ps_tile("ps_o")
                for j, (n0, nsz) in enumerate(HALVES):
                    for mch in range(NCH):
                        mrows = _rows(mch)
                        nc.tensor.matmul(
                            po[:, j, 0:nsz],
                            lhsT=v_nat[0:mrows, mch, h, :],  # M=128: FWL
                            rhs=expS[hh][0:mrows, mch, j, 0:nsz],
                            start=(mch == 0), stop=(mch == NCH - 1),
                        )
                arow = hh * D
                nc.vector.tensor_copy(aoU[arow:arow + D, hp, :, :],
                                      po[0:D, :, 0:F0])
                dstage = spool.tile([1, 2, F0], f32, name="dstage")
                nc.vector.tensor_copy(dstage[0:1, :, :], po[D:D + 1, :, 0:F0])
                nc.gpsimd.dma_start(dsum[h:h + 1, :, :], dstage[0:1, :, :])

        drecip = spool.tile([12, 2, F0], bf16, name="drecip")
        with nc.allow_low_precision(reason="softmax recip bcast via bf16 matmul"):
            nc.vector.reciprocal(drecip[0:Heff, :, :], dsum[0:Heff, :, :])
        pending = (aoT, aoU, drecip, it, small)

    emit_tail(pending)


_GRAPH = None


def _get_graph():
    global _GRAPH
    if _GRAPH is None:
        nc = bacc.Bacc("TRN2", target_bir_lowering=False, debug=False,
                       num_devices=NCORES)
        x_ext = nc.dram_tensor("x", [ITEMS, N, C], f32, kind="ExternalInput").ap()
        wq_ext = nc.dram_tensor("wq", [C, 3 * C], bf16, kind="ExternalInput").ap()
        pw_ext = nc.dram_tensor("pw", [C, C], bf16, kind="ExternalInput").ap()
        pb_ext = nc.dram_tensor("pb", [C], f32r, kind="ExternalInput").ap()
        sel_ext = nc.dram_tensor("sel", [12, CCH, P], bf16, kind="ExternalInput").ap()
        out_ext = nc.dram_tensor("out", [ITEMS, N, C], f32, kind="ExternalOutput").ap()
        with tile.TileContext(nc) as tc:
            with ExitStack() as ctx:
                _emit(ctx, tc, x_ext, wq_ext, pw_ext, pb_ext, sel_ext, out_ext)
        nc.finalize()
        _GRAPH = nc
    return _GRAPH


LAST_RESULTS = None


def kernel(x, qkv_w, proj_w, proj_b, _trace=False):
    global LAST_RESULTS
    x = np.asarray(x, dtype=np.float32)
    wq = np.array(qkv_w, dtype=np.float32)          # copy; rows 0:C are q
    wq[0:C] *= D ** -0.5                            # fold attention scale into Wq
    wqT = np.ascontiguousarray(wq.T).astype(ml_dtypes.bfloat16)   # [C, 3C]
    pwT = np.ascontiguousarray(
        np.asarray(proj_w, dtype=np.float32).T).astype(ml_dtypes.bfloat16)
    pb = np.ascontiguousarray(np.asarray(proj_b, dtype=np.float32))
    sel_np = np.zeros((12, CCH, P), dtype=ml_dtypes.bfloat16)
    for c in range(CCH):
        sel_np[2 * c, c, 0:D] = 1
        sel_np[2 * c + 1, c, D:P] = 1

    nc = _get_graph()
    in_maps = []
    half = x.shape[0] // 2  # 32
    per = half // NCORES    # 4
    for c in range(NCORES):
        xs = np.concatenate(
            [x[per * c:per * (c + 1)], x[half + per * c:half + per * (c + 1)]],
            axis=0,
        )
        in_maps.append({
            "x": np.ascontiguousarray(xs),
            "wq": wqT,
            "pw": pwT,
            "pb": pb,
            "sel": sel_np,
        })

    res = run_bass_kernel_spmd(nc, in_maps, core_ids=list(range(NCORES)),
                               trace=_trace)
    LAST_RESULTS = res

    out = np.empty((x.shape[0], N, C), dtype=np.float32)
    for c in range(NCORES):
        o = res.results[c]["out"]
        out[per * c:per * (c + 1)] = o[0:per]
        out[half + per * c:half + per * (c + 1)] = o[per:2 * per]
    return out


# revision 18
# speedup vs baseline: 1.1785x; 1.0865x over previous
"""Sparse multi-head attention (ViT-style, 577 tokens, 12 heads) on 8 TRN2
NeuronCores.

Sharding: pure data-parallel over batch. Each core gets 8 of the 64 batch
items: 4 from the "large" half (full 12-head attention) and 4 from the
"small" half (compressed: heads 6..11 of q/k/v are statically zero, so only
6 heads + a 384x384 projection are computed). Co-sharding large/small
halves balances per-core compute. No collectives are needed.

Per-item dataflow (everything stays in the transposed domain so no
intermediate ever needs a device transpose except the initial x -> xT):

  x[577,768] --PE-transpose--> xT[c,n]
  qT,kT[o,n] = Wqkv^T-stationary matmuls over xT     (q pre-scaled by D^-0.5)
  v[n,o]     = xT-stationary matmuls over Wv^T, plus a ones column (aug)
  S^T[m,n]   = kT-stationary over qT (per head, K=64)
  P^T        = exp(S^T)  (scalar engine, PSUM->SBUF, bf16; softmax max-shift
               skipped: logits are O(1) by construction)
  aoT[d,n]   = v_aug^T @ P^T  -> row 64 holds the softmax denominators
  normalize via reciprocal + K=1 broadcast matmul
  y[n,oc]    = aoT-stationary over proj_w^T, + bias, DMA out.

Matmuls run as float32r (full PE rate at free-dim >= 256) on fp32 data;
only P^T and v are bf16 (flash-attention-style precision).
"""

import ml_dtypes
import numpy as np
from contextlib import ExitStack

import concourse.bass as bass
import concourse.tile as tile
from concourse import bacc, mybir
from concourse import bass2jax as _b2j
from concourse.bass_utils import run_bass_kernel_spmd
from concourse.masks import make_identity


def _run_bass_via_pjrt_presharded(nc, in_maps, n_cores):
    """Drop-in replacement for bass2jax.run_bass_via_pjrt (multi-core path).

    The stock version concatenates per-core inputs into one host array and
    lets jax reshard it onto the mesh; on the neuron PJRT backend that
    resharding lowers to a compiled "scatter" program which, for ~100MB
    inputs, dies in neuronx-cc codegen (16-bit semaphore_wait_value
    overflow). Here each per-core shard is device_put directly onto its
    device and the global array is assembled zero-copy, so the jitted body
    sees correctly-sharded operands and no data-movement program exists.
    """
    import jax

    _b2j.install_neuronx_cc_hook()
    assert nc.dbg_addr is None and nc.partition_id_tensor is None

    from jax.experimental.shard_map import shard_map
    from jax.sharding import Mesh, NamedSharding, PartitionSpec

    in_names, out_names, out_avals, zero_shapes = [], [], [], []
    for alloc in nc.m.functions[0].allocations:
        if not isinstance(alloc, mybir.MemoryLocationSet):
            continue
        name = alloc.memorylocations[0].name
        if alloc.kind == "ExternalInput":
            in_names.append(name)
        elif alloc.kind == "ExternalOutput":
            shape = tuple(alloc.tensor_shape)
            dtype = mybir.dt.np(alloc.dtype)
            out_names.append(name)
            out_avals.append(jax.core.ShapedArray(shape, dtype))
            zero_shapes.append((shape, dtype))
    n_params = len(in_names)
    n_outs = len(out_names)
    all_names = in_names + out_names
    donate = tuple(range(n_params, n_params + n_outs))

    def _body(*args):
        outs = _b2j._bass_exec_p.bind(
            *args,
            out_avals=tuple(out_avals),
            in_names=tuple(all_names),
            out_names=tuple(out_names),
            lowering_input_output_aliases=(),
            sim_require_finite=True,
            sim_require_nnan=True,
            nc=nc,
        )
        return tuple(outs)

    devices = jax.devices()[:n_cores]
    mesh = Mesh(np.asarray(devices), ("core",))
    sharding = NamedSharding(mesh, PartitionSpec("core"))

    def make_global(shards):
        s0 = np.asarray(shards[0])
        gshape = (n_cores * s0.shape[0], *s0.shape[1:])
        parts = [
            jax.device_put(np.ascontiguousarray(shards[c]), devices[c])
            for c in range(n_cores)
        ]
        return jax.make_array_from_single_device_arrays(gshape, sharding, parts)

    global_ins = [make_global([m[nm] for m in in_maps]) for nm in in_names]
    global_zeros = [
        make_global([np.zeros(shape, dtype)] * n_cores)
        for shape, dtype in zero_shapes
    ]

    sharded = jax.jit(
        shard_map(_body, mesh=mesh, in_specs=(PartitionSpec("core"),) * (n_params + n_outs),
                  out_specs=(PartitionSpec("core"),) * n_outs, check_rep=False),
        donate_argnums=donate,
        keep_unused=True,
    )
    out_arrs = sharded(*global_ins, *global_zeros)

    results = [dict() for _ in range(n_cores)]
    for i, name in enumerate(out_names):
        arr = out_arrs[i]
        per = {s.index[0].start or 0: np.asarray(s.data) for s in arr.addressable_shards}
        step = out_avals[i].shape[0]
        for c in range(n_cores):
            results[c][name] = per[c * step]
    return results


def _patched_run_bass_via_pjrt(nc, in_maps, n_cores):
    if n_cores > 1 and nc.partition_id_tensor is None and nc.dbg_addr is None:
        return _run_bass_via_pjrt_presharded(nc, in_maps, n_cores)
    return _orig_run_bass_via_pjrt(nc, in_maps, n_cores)


_orig_run_bass_via_pjrt = _b2j.run_bass_via_pjrt
_b2j.run_bass_via_pjrt = _patched_run_bass_via_pjrt

P = 128
N = 577
C = 768
H = 12
D = 64
NCH = 5           # n (token) chunks: 4*128 + 65
CCH = 6           # c chunks: 768 / 128
NTAIL = N - 4 * P  # 65
F0, F1 = 290, 288  # n free-dim halves, padded n=578: fp32r needs EVEN free sizes
HALVES = ((0, F0), (F0, F1))
ITEMS = 8
NCORES = 8

f32 = mybir.dt.float32
f32r = mybir.dt.float32r
bf16 = mybir.dt.bfloat16


def _rows(nch):
    return NTAIL if nch == NCH - 1 else P


def _mcols(nch):
    """lhsT column count for an n-chunk: pad the 65-tail to 66 (even M is
    measurably faster on the PE); the extra output partition is discarded."""
    return NTAIL + 1 if nch == NCH - 1 else P


def _emit(ctx, tc, x_ext, wq_ext, pw_ext, pb_ext, sel_ext, out_ext):
    nc = tc.nc

    const_pool = ctx.enter_context(tc.tile_pool(name="const", bufs=1))
    wpool = ctx.enter_context(tc.tile_pool(name="weights", bufs=1))
    xpool = ctx.enter_context(tc.tile_pool(name="xchunk", bufs=1))
    xtpool = ctx.enter_context(tc.tile_pool(name="xt", bufs=2))
    qkpool = ctx.enter_context(tc.tile_pool(name="qkt", bufs=2))
    vpool = ctx.enter_context(tc.tile_pool(name="vnat", bufs=2))
    epool = ctx.enter_context(tc.tile_pool(name="exps", bufs=4))
    aopool = ctx.enter_context(tc.tile_pool(name="aot", bufs=2))
    ypool = ctx.enter_context(tc.tile_pool(name="ychunk", bufs=3))
    spool = ctx.enter_context(tc.tile_pool(name="norm", bufs=2))
    aoupool = ctx.enter_context(tc.tile_pool(name="aou", bufs=2))
    # PSUM: 3x 2-bank slots (scores pairs + AV out) + 2x 1-bank slots
    # (qkv / proj / transpose / bcast staging) = 8 banks.
    ps2 = ctx.enter_context(tc.tile_pool(name="ps2", bufs=3, space="PSUM"))
    ps1 = ctx.enter_context(tc.tile_pool(name="ps1", bufs=2, space="PSUM"))

    def ps2_tile(name):
        return ps2.tile([P, 2, 512], f32, tag="ps2", name=name)

    def ps1_tile(name):
        return ps1.tile([P, 512], f32, tag="ps1", name=name)

    # ---- constants / weights (resident) ----
    ident = const_pool.tile([P, P], f32, name="ident")
    make_identity(nc, ident)

    ones_f32 = const_pool.tile([1, P], f32, name="ones_f32")
    nc.gpsimd.memset(ones_f32[:], 1.0)
    ones_row = const_pool.tile([1, P], f32r, name="ones_row")
    nc.vector.tensor_copy(ones_row[:], ones_f32[:])

    sel = const_pool.tile([12, CCH, P], bf16, name="sel")
    nc.sync.dma_start(sel[:], sel_ext[:])

    wq_sb = wpool.tile([P, CCH, 3 * C], bf16, name="wq_sb")
    nc.sync.dma_start(wq_sb[:], wq_ext.rearrange("(co p) o -> p co o", p=P))
    pw_sb = wpool.tile([P, CCH, C], bf16, name="pw_sb")
    nc.sync.dma_start(pw_sb[:], pw_ext.rearrange("(co p) o -> p co o", p=P))
    pb_sb = const_pool.tile([1, C], f32r, name="pb_sb")
    nc.sync.dma_start(pb_sb[:], pb_ext[None, :])

    # bias broadcast across partitions: [128, 768] = ones[128,1] @ pb[1,768]
    bias_sb = wpool.tile([P, C], f32, name="bias_sb")
    for j in range(2):
        psb0 = ps1_tile("ps_bias")
        nc.tensor.matmul(
            psb0[:, 0:384],
            lhsT=ones_row[0:1, :],
            rhs=pb_sb[0:1, j * 384:(j + 1) * 384],
            start=True, stop=True,
        )
        nc.vector.tensor_copy(bias_sb[:, j * 384:(j + 1) * 384], psb0[:, 0:384])

    # ---- per-item pipeline (1-item software pipeline for the tail) ----
    def emit_tail(st):
        aoT, aoU, drecip, it, small = st
        Heff = H // 2 if small else H
        CCH_ao = CCH // 2 if small else CCH
        NJ = 1 if small else 2
        for c in range(Heff // 2):
            for j, (n0, nsz) in enumerate(HALVES):
                pbc = ps1_tile("ps_bc")
                nc.tensor.matmul(
                    pbc[:, 0:nsz],
                    lhsT=sel[0:12, c, :],
                    rhs=drecip[0:12, j, 0:nsz],
                    start=True, stop=True,
                )
                nc.vector.tensor_mul(
                    aoT[:, c, n0:n0 + nsz],
                    aoU[:, c, j, 0:nsz],
                    pbc[:, 0:nsz],
                )
        for nch in range(NCH):
            rows = _rows(nch)
            yc = ypool.tile([P, C], f32, name="yc")
            if small:
                nc.gpsimd.memset(yc[0:rows, 384:768], 0.0)
            for j in range(NJ):
                o0 = j * 384
                psy = ps1_tile("ps_y")
                for cc in range(CCH_ao):
                    nc.tensor.matmul(
                        psy[0:_mcols(nch), 0:384],
                        lhsT=aoT[:, cc, nch * P:nch * P + _mcols(nch)],
                        rhs=pw_sb[:, cc, o0:o0 + 384],
                        start=(cc == 0), stop=(cc == CCH_ao - 1),
                    )
                nc.vector.tensor_add(
                    yc[0:rows, o0:o0 + 384],
                    psy[0:rows, 0:384],
                    bias_sb[0:rows, o0:o0 + 384],
                )
            nc.gpsimd.dma_start(out_ext[it, nch * P:nch * P + rows, :],
                                yc[0:rows, :])

    pending = None
    for it in range(ITEMS):
        small = it >= ITEMS // 2
        Heff = H // 2 if small else H
        qk_chunks = ([0, 1, 2, 6, 7, 8] if small else list(range(12)))
        NJ = 1 if small else 2

        # Phase A: load x (2 batched DMAs), transpose to xT [c-part, n-free]
        xT = xtpool.tile([P, CCH, 640], bf16, name="xT")
        nc.gpsimd.memset(xT[:, :, N], 0.0)
        xn = xpool.tile([P, NCH, C], f32, name="xn")
        nc.sync.dma_start(
            xn[:, 0:4, :],
            x_ext[it, 0:4 * P, :].rearrange("(o p) c -> p o c", p=P))
        nc.sync.dma_start(xn[0:NTAIL, 4, :], x_ext[it, 4 * P:N, :])
        for nch in range(NCH):
            rows = _rows(nch)
            for cc in range(CCH):
                pst = ps1_tile("ps_t")
                nc.tensor.transpose(
                    pst[:, 0:rows],
                    xn[0:rows, nch, cc * P:(cc + 1) * P],
                    ident[0:rows, 0:rows],
                )
                nc.vector.tensor_copy(
                    xT[:, cc, nch * P:nch * P + rows], pst[:, 0:rows])

        # Phase B: qT / kT (transposed outputs) for needed o-chunks
        qkT = qkpool.tile([P, 12, N + 1], bf16, name="qkT")
        for oc in qk_chunks:
            wcol = oc * P if oc < 6 else C + (oc - 6) * P
            for j, (n0, nsz) in enumerate(HALVES):
                pqk = ps1_tile("ps_qk")
                for cc in range(CCH):
                    nc.tensor.matmul(
                        pqk[:, 0:nsz],
                        lhsT=wq_sb[:, cc, wcol:wcol + P],
                        rhs=xT[:, cc, n0:n0 + nsz],
                        start=(cc == 0), stop=(cc == CCH - 1),
                    )
                nc.vector.tensor_copy(qkT[:, oc, n0:n0 + nsz], pqk[:, 0:nsz])

        # deferred tail of the previous item, hidden behind this item's qkv
        if pending is not None:
            emit_tail(pending)
            pending = None

        # Phase C: v natural [n-part, (h,d)-free], 128-wide lanes (FWL),
        # ones column at d=64 feeds the softmax denominators.
        v_nat = vpool.tile([P, NCH, H, 2 * D], bf16, name="v_nat")
        nc.gpsimd.memset(v_nat[:, :, :, D:2 * D], 0.0)
        nc.gpsimd.memset(v_nat[:, :, :, D], 1.0)
        for nch in range(NCH):
            rows = _rows(nch)
            for j in range(NJ):
                vcol = 2 * C + j * 384
                pv = ps1_tile("ps_v")
                for cc in range(CCH):
                    nc.tensor.matmul(
                        pv[0:_mcols(nch), 0:384],
                        lhsT=xT[:, cc, nch * P:nch * P + _mcols(nch)],
                        rhs=wq_sb[:, cc, vcol:vcol + 384],
                        start=(cc == 0), stop=(cc == CCH - 1),
                    )
                nc.vector.tensor_copy(
                    v_nat[0:rows, nch, j * 6:(j + 1) * 6, 0:D],
                    pv[0:rows, 0:384].rearrange("p (h d) -> p h d", h=6),
                )

        # Phase D/E: 2-stage pair pipeline — scores+exp of pair p overlap
        # AV of pair p-1, so the scalar engine's exp stream stays ahead of
        # the PE's AV consumption.
        aoT = aopool.tile([P, CCH, N + 1], bf16, name="aoT")
        aoU = aoupool.tile([P, CCH, 2, F0], bf16, name="aoU")
        dsum = spool.tile([12, 2, F0], f32, name="dsum")

        def emit_scores(hp):
            kch = 6 + hp
            expS = [None, None]
            for hh in (0, 1):
                expS[hh] = epool.tile([P, NCH, 2, F0], bf16, tag="expS",
                                      name=f"expS{hh}")
            for mch in range(NCH):
                mrows = _rows(mch)
                pse = [None, None]
                for hh in (0, 1):
                    hrow = hh * D
                    pse[hh] = ps2_tile(f"ps_s{hh}")
                    for j, (n0, nsz) in enumerate(HALVES):
                        nc.tensor.matmul(
                            pse[hh][0:_mcols(mch), j, 0:nsz],
                            lhsT=qkT[hrow:hrow + D, kch, mch * P:mch * P + _mcols(mch)],
                            rhs=qkT[hrow:hrow + D, hp, n0:n0 + nsz],
                            start=True, stop=True,
                            tile_position=(hrow, 0),
                        )
                for hh in (0, 1):
                    nc.scalar.activation(
                        expS[hh][0:mrows, mch, :, :],
                        pse[hh][0:mrows, :, 0:F0],
                        mybir.ActivationFunctionType.Exp,
                    )
            return expS

        def emit_av(hp, expS):
            for hh in (0, 1):
                h = 2 * hp + hh
                po = ps2_tile("ps_o")
                for j, (n0, nsz) in enumerate(HALVES):
                    for mch in range(NCH):
                        mrows = _rows(mch)
                        nc.tensor.matmul(
                            po[:, j, 0:nsz],
                            lhsT=v_nat[0:mrows, mch, h, :],
                            rhs=expS[hh][0:mrows, mch, j, 0:nsz],
                            start=(mch == 0), stop=(mch == NCH - 1),
                        )
                arow = hh * D
                nc.vector.tensor_copy(aoU[arow:arow + D, hp, :, :],
                                      po[0:D, :, 0:F0])
                dstage = spool.tile([1, 2, F0], f32, name="dstage")
                nc.vector.tensor_copy(dstage[0:1, :, :], po[D:D + 1, :, 0:F0])
                nc.gpsimd.dma_start(dsum[h:h + 1, :, :], dstage[0:1, :, :])

        prev = None
        for hp in range(Heff // 2):
            expS = emit_scores(hp)
            if prev is not None:
                emit_av(prev[0], prev[1])
            prev = (hp, expS)
        emit_av(prev[0], prev[1])

        drecip = spool.tile([12, 2, F0], bf16, name="drecip")
        with nc.allow_low_precision(reason="softmax recip bcast via bf16 matmul"):
            nc.vector.reciprocal(drecip[0:Heff, :, :], dsum[0:Heff, :, :])
        pending = (aoT, aoU, drecip, it, small)

    emit_tail(pending)


_GRAPH = None


def _get_graph():
    global _GRAPH
    if _GRAPH is None:
        nc = bacc.Bacc("TRN2", target_bir_lowering=False, debug=False,
                       num_devices=NCORES)
        x_ext = nc.dram_tensor("x", [ITEMS, N, C], f32, kind="ExternalInput").ap()
        wq_ext = nc.dram_tensor("wq", [C, 3 * C], bf16, kind="ExternalInput").ap()
        pw_ext = nc.dram_tensor("pw", [C, C], bf16, kind="ExternalInput").ap()
        pb_ext = nc.dram_tensor("pb", [C], f32r, kind="ExternalInput").ap()
        sel_ext = nc.dram_tensor("sel", [12, CCH, P], bf16, kind="ExternalInput").ap()
        out_ext = nc.dram_tensor("out", [ITEMS, N, C], f32, kind="ExternalOutput").ap()
        with tile.TileContext(nc) as tc:
            with ExitStack() as ctx:
                _emit(ctx, tc, x_ext, wq_ext, pw_ext, pb_ext, sel_ext, out_ext)
        nc.finalize()
        _GRAPH = nc
    return _GRAPH


LAST_RESULTS = None


def kernel(x, qkv_w, proj_w, proj_b, _trace=False):
    global LAST_RESULTS
    x = np.asarray(x, dtype=np.float32)
    wq = np.array(qkv_w, dtype=np.float32)          # copy; rows 0:C are q
    wq[0:C] *= D ** -0.5                            # fold attention scale into Wq
    wqT = np.ascontiguousarray(wq.T).astype(ml_dtypes.bfloat16)   # [C, 3C]
    pwT = np.ascontiguousarray(
        np.asarray(proj_w, dtype=np.float32).T).astype(ml_dtypes.bfloat16)
    pb = np.ascontiguousarray(np.asarray(proj_b, dtype=np.float32))
    sel_np = np.zeros((12, CCH, P), dtype=ml_dtypes.bfloat16)
    for c in range(CCH):
        sel_np[2 * c, c, 0:D] = 1
        sel_np[2 * c + 1, c, D:P] = 1

    nc = _get_graph()
    in_maps = []
    half = x.shape[0] // 2  # 32
    per = half // NCORES    # 4
    for c in range(NCORES):
        xs = np.concatenate(
            [x[per * c:per * (c + 1)], x[half + per * c:half + per * (c + 1)]],
            axis=0,
        )
        in_maps.append({
            "x": np.ascontiguousarray(xs),
            "wq": wqT,
            "pw": pwT,
            "pb": pb,
            "sel": sel_np,
        })

    res = run_bass_kernel_spmd(nc, in_maps, core_ids=list(range(NCORES)),
                               trace=_trace)
    LAST_RESULTS = res

    out = np.empty((x.shape[0], N, C), dtype=np.float32)
    for c in range(NCORES):
        o = res.results[c]["out"]
        out[per * c:per * (c + 1)] = o[0:per]
        out[half + per * c:half + per * (c + 1)] = o[per:2 * per]
    return out
